# revision 2
# baseline (speedup 1.0000x reference)
"""Sparse MoE routing kernel for trn2 (8 NeuronCores, expert-parallel).

Reference semantics:
    logits = x @ router_w; p = softmax(logits); top2 renormalized weights
    out[t] = sum_{e in top2(t)} we[t,e] * (silu(x@w1[e]) * (x@v1[e])) @ w2[e]

The reference computes every expert on every token densely, but only the
top-2 experts per token contribute to the output.  This kernel exploits
that sparsity: routing (a [T,8] matmul + top-2, 0.3% of the FLOPs) runs
on the host, which then gathers each expert's assigned tokens into a
compact padded buffer.  Core r holds expert r's weights and runs the
gated FFN only on its own tokens (~T/4 of them), in fp16 (PSUM
accumulation stays fp32; measured end-to-end rel err ~1e-3, tolerance
2e-2).  The host applies the per-token top-2 combine weights and
scatter-adds the per-expert partial outputs — each token appears in
exactly two experts' buffers, so no device collective is needed.

Per-core device work drops from 3 dense T-token matmuls (fp32r) to 3
C-token matmuls (fp16), C = max expert load padded to 128.
"""

import math
import os

import numpy as np

import concourse.mybir as mybir
import concourse.tile as tile
from concourse import bacc
from concourse.bass_utils import run_bass_kernel_spmd

P = 128
N_CORES = 8
F32 = mybir.dt.float32
F16 = mybir.dt.float16
ACTF = mybir.ActivationFunctionType


def _install_trace_hook_if_requested():
    """Optional: enables NTFF profiling when BASS_TRACE=1 (dev only)."""
    if os.environ.get("BASS_TRACE") != "1":
        return
    import sys
    import types

    if "antenv.axon_hooks" in sys.modules:
        return
    mod = types.ModuleType("antenv.axon_hooks")
    state = {"hook": None}
    mod.set_axon_ntff_profile_hook = lambda h: state.__setitem__("hook", h)
    mod.get_axon_ntff_profile_hook = lambda: state["hook"]
    sys.modules["antenv.axon_hooks"] = mod
    try:
        from trn_agent_boot.trn_boot import _ntff_profile_via_ctypes

        mod.set_axon_ntff_profile_hook(
            _ntff_profile_via_ctypes("/opt/axon/libaxon_pjrt.so")
        )
    except Exception:
        pass


def build(C, D, F, chunks):
    """Per-core program: gated FFN on C gathered tokens, fp16 matmuls.

    xT [D, C] fp16 (token on free axis); weights host-swizzled so every
    DMA line is per-partition contiguous:
      wv[f, p, d, 0/1, j]  = w1/v1[d*P+p, f*P+j]
      w2s[dt, p, f, j]     = w2[f*P+p, dt*P+j]
    outT [D, C] fp32 = (silu(w1.T x) * (v1.T x)) contracted with w2.
    `chunks` split C into <=512-column pieces (PSUM bank = 512 fp32).
    """
    DC = D // P   # contraction chunks over D
    FT = F // P   # partition tiles of F
    DT = D // P   # output d tiles
    assert sum(chunks) == C and all(s <= 512 for s in chunks)

    nc = bacc.Bacc("TRN2", target_bir_lowering=False, debug=False,
                   num_devices=N_CORES)

    xT = nc.dram_tensor("xT", [D, C], F16, kind="ExternalInput")
    wv = nc.dram_tensor("wv", [FT, P, DC, 2, P], F16, kind="ExternalInput")
    w2s = nc.dram_tensor("w2s", [DT, P, FT, P], F16, kind="ExternalInput")
    outT = nc.dram_tensor("outT", [DT, P, C], F32, kind="ExternalOutput")

    offs = []
    o = 0
    for s in chunks:
        offs.append((o, s))
        o += s

    with tile.TileContext(nc) as tc:
        with (
            tc.tile_pool(name="xpool", bufs=1) as xpool,
            tc.tile_pool(name="wpool", bufs=5) as wpool,
            tc.tile_pool(name="w2pool", bufs=4) as w2pool,
            tc.tile_pool(name="gpool", bufs=FT) as gpool,
            tc.tile_pool(name="spool", bufs=3) as spool,
            tc.tile_pool(name="opool", bufs=3) as opool,
            tc.tile_pool(name="ph", bufs=2, space="PSUM") as ph_pool,
            tc.tile_pool(name="pv", bufs=2, space="PSUM") as pv_pool,
            tc.tile_pool(name="po", bufs=3, space="PSUM") as po_pool,
        ):
            # gathered activations: all DC [P, C] tiles stay resident
            x_tiles = []
            for d in range(DC):
                xt = xpool.tile([P, C], F16, name=f"x{d}")
                eng = (nc.sync, nc.scalar, nc.gpsimd)[d % 3]
                eng.dma_start(xt[:], xT[d * P:(d + 1) * P, :])
                x_tiles.append(xt)

            # phase 1: g[f] = silu(w1.T x) * (v1.T x), kept in SBUF (fp16)
            gts = []
            for f in range(FT):
                wv_cb = wpool.tile([P, DC, 2, P], F16, name="wv_cb")
                eng = nc.sync if f % 2 == 0 else nc.gpsimd
                eng.dma_start(wv_cb[:], wv[f])
                gt = gpool.tile([P, C], F16, name="gt")
                for (o, s) in offs:
                    ps_h = ph_pool.tile([P, 512], F32, name="ps_h")[:, :s]
                    for d in range(DC):
                        nc.tensor.matmul(ps_h[:], wv_cb[:, d, 0, :],
                                         x_tiles[d][:, o:o + s],
                                         start=(d == 0), stop=(d == DC - 1))
                    ps_v = pv_pool.tile([P, 512], F32, name="ps_v")[:, :s]
                    for d in range(DC):
                        nc.tensor.matmul(ps_v[:], wv_cb[:, d, 1, :],
                                         x_tiles[d][:, o:o + s],
                                         start=(d == 0), stop=(d == DC - 1))
                    sl = spool.tile([P, 512], F16, name="sl")[:, :s]
                    nc.scalar.activation(sl[:], ps_h[:], ACTF.Silu)
                    nc.vector.tensor_mul(gt[:, o:o + s], sl[:], ps_v[:])
                gts.append(gt)

            # phase 2: outT[dt] = sum_f w2[f,dt].T g[f]
            for dt in range(DT):
                w2_cb = w2pool.tile([P, FT, P], F16, name="w2_cb")
                eng = nc.sync if dt % 2 == 0 else nc.gpsimd
                eng.dma_start(w2_cb[:], w2s[dt])
                for (o, s) in offs:
                    ps_o = po_pool.tile([P, 512], F32, name="ps_o")[:, :s]
                    for f in range(FT):
                        nc.tensor.matmul(ps_o[:], w2_cb[:, f, :],
                                         gts[f][:, o:o + s],
                                         start=(f == 0), stop=(f == FT - 1))
                    ob = opool.tile([P, 512], F32, name="ob")[:, :s]
                    nc.vector.tensor_copy(ob[:], ps_o[:])
                    nc.scalar.dma_start(outT[dt, :, o:o + s], ob[:])

    nc.finalize()
    return nc


_CACHE = {}
LAST_RESULTS = None


def _get_nc(C, D, F, chunks):
    key = (C, D, F, chunks)
    if key not in _CACHE:
        _CACHE[key] = build(C, D, F, chunks)
    return _CACHE[key]


def _split_chunks(C):
    """Split C into equal-ish pieces of at most 512 columns."""
    n = math.ceil(C / 512)
    base = C // n
    rem = C - base * n
    return tuple(base + (1 if i < rem else 0) for i in range(n))


def run_moe(hidden_states, router_w, w1, v1, w2):
    global LAST_RESULTS
    _install_trace_hook_if_requested()

    B, S, D = hidden_states.shape
    E = router_w.shape[1]
    F = w1.shape[2]
    T = B * S
    DC, FT, DT = D // P, F // P, D // P

    x = np.ascontiguousarray(hidden_states.reshape(T, D).astype(np.float32))

    # host router: fp64 logits; top-2 renormalized softmax = sigmoid of
    # the logit difference (full-softmax denominator cancels)
    logits = x.astype(np.float64) @ router_w.astype(np.float64)
    top1 = np.argmax(logits, axis=1)
    l1 = logits[np.arange(T), top1]
    masked = logits.copy()
    masked[np.arange(T), top1] = -np.inf
    top2 = np.argmax(masked, axis=1)
    l2 = masked[np.arange(T), top2]
    w_top1 = 1.0 / (1.0 + np.exp(l2 - l1))

    idx, wts = [], []
    for r in range(N_CORES):
        sel1 = top1 == r
        sel2 = top2 == r
        idx_r = np.where(sel1 | sel2)[0]
        we_r = np.where(sel1[idx_r], w_top1[idx_r], 1.0 - w_top1[idx_r])
        idx.append(idx_r)
        wts.append(we_r.astype(np.float32))

    C = max(256, math.ceil(max(len(i) for i in idx) / 128) * 128)
    chunks = _split_chunks(C)
    nc = _get_nc(C, D, F, chunks)

    in_maps = []
    for r in range(N_CORES):
        xg = np.zeros((C, D), dtype=np.float16)
        xg[:len(idx[r])] = x[idx[r]]
        xgT = np.ascontiguousarray(xg.T)
        # swizzle: wv[f, p, i, 0/1, j] = w1/v1[i*P+p, f*P+j]
        w1t = w1[r].astype(np.float16).reshape(DC, P, FT, P).transpose(2, 1, 0, 3)
        v1t = v1[r].astype(np.float16).reshape(DC, P, FT, P).transpose(2, 1, 0, 3)
        wvr = np.ascontiguousarray(np.stack([w1t, v1t], axis=3))
        # w2s[dt, p, i, j] = w2[i*P+p, dt*P+j]
        w2r = np.ascontiguousarray(
            w2[r].astype(np.float16).reshape(FT, P, DT, P).transpose(2, 1, 0, 3))
        in_maps.append({"xT": xgT, "wv": wvr, "w2s": w2r})

    res = run_bass_kernel_spmd(nc, in_maps, core_ids=list(range(N_CORES)))
    LAST_RESULTS = res

    out = np.zeros((T, D), dtype=np.float32)
    for r in range(N_CORES):
        o = res.results[r]["outT"].reshape(D, C)  # [D, C]
        n = len(idx[r])
        out[idx[r]] += wts[r][:, None] * o[:, :n].T
    return out.reshape(B, S, D)


def kernel(hidden_states, router_w, w1, v1, w2):
    return run_moe(hidden_states, router_w, w1, v1, w2)


# revision 4
# speedup vs baseline: 4.5839x; 4.5839x over previous
"""Sparse MoE routing kernel for trn2 (8 NeuronCores, expert-parallel).

Reference semantics:
    logits = x @ router_w; p = softmax(logits); top2 renormalized weights
    out[t] = sum_{e in top2(t)} we[t,e] * (silu(x@w1[e]) * (x@v1[e])) @ w2[e]

The reference computes every expert on every token densely, but only the
top-2 experts per token contribute to the output.  This kernel exploits
that sparsity: routing (a [T,8] matmul + top-2, 0.3% of the FLOPs) runs
on the host, which then gathers each expert's assigned tokens into a
compact padded buffer.  Core r holds expert r's weights and runs the
gated FFN only on its own tokens (~T/4 of them), in fp16 (PSUM
accumulation stays fp32; measured end-to-end rel err ~1e-3, tolerance
2e-2).  The host applies the per-token top-2 combine weights and
scatter-adds the per-expert partial outputs — each token appears in
exactly two experts' buffers, so no device collective is needed.

Per-core device work drops from 3 dense T-token matmuls (fp32r) to 3
C-token matmuls (fp16), C = max expert load padded to 128.
"""

import math
import os

import numpy as np

import concourse.mybir as mybir
import concourse.tile as tile
from concourse import bacc
from concourse.bass_utils import run_bass_kernel_spmd

P = 128
N_CORES = 8
F32 = mybir.dt.float32
F16 = mybir.dt.float16
ACTF = mybir.ActivationFunctionType


def _install_trace_hook_if_requested():
    """Optional: enables NTFF profiling when BASS_TRACE=1 (dev only)."""
    if os.environ.get("BASS_TRACE") != "1":
        return
    import sys
    import types

    if "antenv.axon_hooks" in sys.modules:
        return
    mod = types.ModuleType("antenv.axon_hooks")
    state = {"hook": None}
    mod.set_axon_ntff_profile_hook = lambda h: state.__setitem__("hook", h)
    mod.get_axon_ntff_profile_hook = lambda: state["hook"]
    sys.modules["antenv.axon_hooks"] = mod
    try:
        from trn_agent_boot.trn_boot import _ntff_profile_via_ctypes

        mod.set_axon_ntff_profile_hook(
            _ntff_profile_via_ctypes("/opt/axon/libaxon_pjrt.so")
        )
    except Exception:
        pass


def build(C, D, F, chunks):
    """Per-core program: gated FFN on C gathered tokens, fp16 matmuls.

    xT [D, C] fp16 (token on free axis); weights host-swizzled so every
    DMA line is per-partition contiguous:
      wv[f, p, d, 0/1, j]  = w1/v1[d*P+p, f*P+j]
      w2s[dt, p, f, j]     = w2[f*P+p, dt*P+j]
    outT [D, C] fp32 = (silu(w1.T x) * (v1.T x)) contracted with w2.
    `chunks` split C into <=512-column pieces (PSUM bank = 512 fp32).
    """
    DC = D // P   # contraction chunks over D
    FT = F // P   # partition tiles of F
    DT = D // P   # output d tiles
    assert sum(chunks) == C and all(s <= 512 for s in chunks)

    nc = bacc.Bacc("TRN2", target_bir_lowering=False, debug=False,
                   num_devices=N_CORES)

    xT = nc.dram_tensor("xT", [D, C], F16, kind="ExternalInput")
    wv = nc.dram_tensor("wv", [FT, P, DC, 2, P], F16, kind="ExternalInput")
    w2s = nc.dram_tensor("w2s", [DT, P, FT, P], F16, kind="ExternalInput")
    outT = nc.dram_tensor("outT", [DT, P, C], F32, kind="ExternalOutput")

    offs = []
    o = 0
    for s in chunks:
        offs.append((o, s))
        o += s

    with tile.TileContext(nc) as tc:
        with (
            tc.tile_pool(name="xpool", bufs=D // P) as xpool,
            tc.tile_pool(name="wpool", bufs=5) as wpool,
            tc.tile_pool(name="w2pool", bufs=4) as w2pool,
            tc.tile_pool(name="gpool", bufs=FT) as gpool,
            tc.tile_pool(name="spool", bufs=3) as spool,
            tc.tile_pool(name="opool", bufs=3) as opool,
            tc.tile_pool(name="ph", bufs=2, space="PSUM") as ph_pool,
            tc.tile_pool(name="pv", bufs=2, space="PSUM") as pv_pool,
            tc.tile_pool(name="po", bufs=3, space="PSUM") as po_pool,
        ):
            # gathered activations: all DC [P, C] tiles stay resident
            x_tiles = []
            for d in range(DC):
                xt = xpool.tile([P, C], F16, name="x_sb")
                eng = (nc.sync, nc.scalar, nc.gpsimd)[d % 3]
                eng.dma_start(xt[:], xT[d * P:(d + 1) * P, :])
                x_tiles.append(xt)

            # phase 1: g[f] = silu(w1.T x) * (v1.T x), kept in SBUF (fp16)
            gts = []
            for f in range(FT):
                wv_cb = wpool.tile([P, DC, 2, P], F16, name="wv_cb")
                eng = nc.sync if f % 2 == 0 else nc.gpsimd
                eng.dma_start(wv_cb[:], wv[f])
                gt = gpool.tile([P, C], F16, name="gt")
                for (o, s) in offs:
                    ps_h = ph_pool.tile([P, 512], F32, name="ps_h")[:, :s]
                    for d in range(DC):
                        nc.tensor.matmul(ps_h[:], wv_cb[:, d, 0, :],
                                         x_tiles[d][:, o:o + s],
                                         start=(d == 0), stop=(d == DC - 1))
                    ps_v = pv_pool.tile([P, 512], F32, name="ps_v")[:, :s]
                    for d in range(DC):
                        nc.tensor.matmul(ps_v[:], wv_cb[:, d, 1, :],
                                         x_tiles[d][:, o:o + s],
                                         start=(d == 0), stop=(d == DC - 1))
                    sl = spool.tile([P, 512], F16, name="sl")[:, :s]
                    nc.scalar.activation(sl[:], ps_h[:], ACTF.Silu)
                    nc.vector.tensor_mul(gt[:, o:o + s], sl[:], ps_v[:])
                gts.append(gt)

            # phase 2: outT[dt] = sum_f w2[f,dt].T g[f]
            for dt in range(DT):
                w2_cb = w2pool.tile([P, FT, P], F16, name="w2_cb")
                eng = nc.sync if dt % 2 == 0 else nc.gpsimd
                eng.dma_start(w2_cb[:], w2s[dt])
                for (o, s) in offs:
                    ps_o = po_pool.tile([P, 512], F32, name="ps_o")[:, :s]
                    for f in range(FT):
                        nc.tensor.matmul(ps_o[:], w2_cb[:, f, :],
                                         gts[f][:, o:o + s],
                                         start=(f == 0), stop=(f == FT - 1))
                    ob = opool.tile([P, 512], F32, name="ob")[:, :s]
                    nc.vector.tensor_copy(ob[:], ps_o[:])
                    nc.scalar.dma_start(outT[dt, :, o:o + s], ob[:])

    nc.finalize()
    return nc


_CACHE = {}
LAST_RESULTS = None


def _get_nc(C, D, F, chunks):
    key = (C, D, F, chunks)
    if key not in _CACHE:
        _CACHE[key] = build(C, D, F, chunks)
    return _CACHE[key]


def _split_chunks(C):
    """Split C into equal-ish pieces of at most 512 columns."""
    n = math.ceil(C / 512)
    base = C // n
    rem = C - base * n
    return tuple(base + (1 if i < rem else 0) for i in range(n))


def run_moe(hidden_states, router_w, w1, v1, w2):
    global LAST_RESULTS
    _install_trace_hook_if_requested()

    B, S, D = hidden_states.shape
    E = router_w.shape[1]
    F = w1.shape[2]
    T = B * S
    DC, FT, DT = D // P, F // P, D // P

    x = np.ascontiguousarray(hidden_states.reshape(T, D).astype(np.float32))

    # host router: fp64 logits; top-2 renormalized softmax = sigmoid of
    # the logit difference (full-softmax denominator cancels)
    logits = x.astype(np.float64) @ router_w.astype(np.float64)
    top1 = np.argmax(logits, axis=1)
    l1 = logits[np.arange(T), top1]
    masked = logits.copy()
    masked[np.arange(T), top1] = -np.inf
    top2 = np.argmax(masked, axis=1)
    l2 = masked[np.arange(T), top2]
    w_top1 = 1.0 / (1.0 + np.exp(l2 - l1))

    idx, wts = [], []
    for r in range(N_CORES):
        sel1 = top1 == r
        sel2 = top2 == r
        idx_r = np.where(sel1 | sel2)[0]
        we_r = np.where(sel1[idx_r], w_top1[idx_r], 1.0 - w_top1[idx_r])
        idx.append(idx_r)
        wts.append(we_r.astype(np.float32))

    C = max(256, math.ceil(max(len(i) for i in idx) / 128) * 128)
    chunks = _split_chunks(C)
    nc = _get_nc(C, D, F, chunks)

    in_maps = []
    for r in range(N_CORES):
        xg = np.zeros((C, D), dtype=np.float16)
        xg[:len(idx[r])] = x[idx[r]]
        xgT = np.ascontiguousarray(xg.T)
        # swizzle: wv[f, p, i, 0/1, j] = w1/v1[i*P+p, f*P+j]
        w1t = w1[r].astype(np.float16).reshape(DC, P, FT, P).transpose(2, 1, 0, 3)
        v1t = v1[r].astype(np.float16).reshape(DC, P, FT, P).transpose(2, 1, 0, 3)
        wvr = np.ascontiguousarray(np.stack([w1t, v1t], axis=3))
        # w2s[dt, p, i, j] = w2[i*P+p, dt*P+j]
        w2r = np.ascontiguousarray(
            w2[r].astype(np.float16).reshape(FT, P, DT, P).transpose(2, 1, 0, 3))
        in_maps.append({"xT": xgT, "wv": wvr, "w2s": w2r})

    res = run_bass_kernel_spmd(nc, in_maps, core_ids=list(range(N_CORES)))
    LAST_RESULTS = res

    out = np.zeros((T, D), dtype=np.float32)
    for r in range(N_CORES):
        o = res.results[r]["outT"].reshape(D, C)  # [D, C]
        n = len(idx[r])
        out[idx[r]] += wts[r][:, None] * o[:, :n].T
    return out.reshape(B, S, D)


def kernel(hidden_states, router_w, w1, v1, w2):
    return run_moe(hidden_states, router_w, w1, v1, w2)


# revision 6
# speedup vs baseline: 5.1828x; 1.1306x over previous
"""Sparse MoE routing kernel for trn2 (8 NeuronCores, expert-parallel).

Reference semantics:
    logits = x @ router_w; p = softmax(logits); top2 renormalized weights
    out[t] = sum_{e in top2(t)} we[t,e] * (silu(x@w1[e]) * (x@v1[e])) @ w2[e]

The reference computes every expert on every token densely, but only the
top-2 experts per token contribute to the output.  This kernel exploits
that sparsity: routing (a [T,8] matmul + top-2, 0.3% of the FLOPs) runs
on the host, which then gathers each expert's assigned tokens into a
compact padded buffer.  Core r holds expert r's weights and runs the
gated FFN only on its own tokens (~T/4 of them), in fp16 (PSUM
accumulation stays fp32; measured end-to-end rel err ~1e-3, tolerance
2e-2).  The host applies the per-token top-2 combine weights and
scatter-adds the per-expert partial outputs — each token appears in
exactly two experts' buffers, so no device collective is needed.

Per-core device work drops from 3 dense T-token matmuls (fp32r) to 3
C-token matmuls (fp16), C = max expert load padded to 128.
"""

import math
import os

import numpy as np

import concourse.mybir as mybir
import concourse.tile as tile
from concourse import bacc
from concourse.bass_utils import run_bass_kernel_spmd

P = 128
N_CORES = 8
F32 = mybir.dt.float32
F16 = mybir.dt.float16
ACTF = mybir.ActivationFunctionType


def _install_trace_hook_if_requested():
    """Optional: enables NTFF profiling when BASS_TRACE=1 (dev only)."""
    if os.environ.get("BASS_TRACE") != "1":
        return
    import sys
    import types

    if "antenv.axon_hooks" in sys.modules:
        return
    mod = types.ModuleType("antenv.axon_hooks")
    state = {"hook": None}
    mod.set_axon_ntff_profile_hook = lambda h: state.__setitem__("hook", h)
    mod.get_axon_ntff_profile_hook = lambda: state["hook"]
    sys.modules["antenv.axon_hooks"] = mod
    try:
        from trn_agent_boot.trn_boot import _ntff_profile_via_ctypes

        mod.set_axon_ntff_profile_hook(
            _ntff_profile_via_ctypes("/opt/axon/libaxon_pjrt.so")
        )
    except Exception:
        pass


def build(C, D, F, chunks):
    """Per-core program: gated FFN on C gathered tokens, fp16 matmuls.

    xT [D, C] fp16 (token on free axis); weights host-swizzled so every
    DMA line is per-partition contiguous:
      wv[f, p, d, 0/1, j]  = w1/v1[d*P+p, f*P+j]
      w2s[dt, p, f, j]     = w2[f*P+p, dt*P+j]
    outT [D, C] fp32 = (silu(w1.T x) * (v1.T x)) contracted with w2.
    `chunks` split C into <=512-column pieces (PSUM bank = 512 fp32).
    """
    DC = D // P   # contraction chunks over D
    FT = F // P   # partition tiles of F
    DT = D // P   # output d tiles
    assert sum(chunks) == C and all(s <= 512 for s in chunks)

    nc = bacc.Bacc("TRN2", target_bir_lowering=False, debug=False,
                   num_devices=N_CORES)

    xT = nc.dram_tensor("xT", [D, C], F16, kind="ExternalInput")
    wv = nc.dram_tensor("wv", [FT, P, DC, 2, P], F16, kind="ExternalInput")
    w2s = nc.dram_tensor("w2s", [DT, P, FT, P], F16, kind="ExternalInput")
    outT = nc.dram_tensor("outT", [DT, P, C], F32, kind="ExternalOutput")

    offs = []
    o = 0
    for s in chunks:
        offs.append((o, s))
        o += s

    with tile.TileContext(nc) as tc:
        with (
            tc.tile_pool(name="xpool", bufs=D // P) as xpool,
            tc.tile_pool(name="wpool", bufs=5) as wpool,
            tc.tile_pool(name="w2pool", bufs=4) as w2pool,
            tc.tile_pool(name="gpool", bufs=FT) as gpool,
            tc.tile_pool(name="spool", bufs=3) as spool,
            tc.tile_pool(name="opool", bufs=3) as opool,
            tc.tile_pool(name="ph", bufs=2, space="PSUM") as ph_pool,
            tc.tile_pool(name="pv", bufs=2, space="PSUM") as pv_pool,
            tc.tile_pool(name="po", bufs=3, space="PSUM") as po_pool,
        ):
            # gathered activations: all DC [P, C] tiles stay resident.
            # Column-chunk loads so the first accumulation group's slice
            # lands (and the PE starts) before the full x is resident.
            x_tiles = [xpool.tile([P, C], F16, name="x_sb")
                       for _ in range(DC)]
            for (o, s) in offs:
                for d in range(DC):
                    eng = (nc.sync, nc.scalar, nc.gpsimd)[d % 3]
                    eng.dma_start(x_tiles[d][:, o:o + s],
                                  xT[d * P:(d + 1) * P, o:o + s])

            # phase 1: g[f] = silu(w1.T x) * (v1.T x), kept in SBUF (fp16)
            gts = []
            for f in range(FT):
                wv_cb = wpool.tile([P, DC, 2, P], F16, name="wv_cb")
                eng = nc.sync if f % 2 == 0 else nc.gpsimd
                eng.dma_start(wv_cb[:], wv[f])
                gt = gpool.tile([P, C], F16, name="gt")
                for (o, s) in offs:
                    ps_h = ph_pool.tile([P, 512], F32, name="ps_h")[:, :s]
                    for d in range(DC):
                        nc.tensor.matmul(ps_h[:], wv_cb[:, d, 0, :],
                                         x_tiles[d][:, o:o + s],
                                         start=(d == 0), stop=(d == DC - 1))
                    ps_v = pv_pool.tile([P, 512], F32, name="ps_v")[:, :s]
                    for d in range(DC):
                        nc.tensor.matmul(ps_v[:], wv_cb[:, d, 1, :],
                                         x_tiles[d][:, o:o + s],
                                         start=(d == 0), stop=(d == DC - 1))
                    sl = spool.tile([P, 512], F16, name="sl")[:, :s]
                    nc.scalar.activation(sl[:], ps_h[:], ACTF.Silu)
                    nc.vector.tensor_mul(gt[:, o:o + s], sl[:], ps_v[:])
                gts.append(gt)

            # phase 2: outT[dt] = sum_f w2[f,dt].T g[f]
            for dt in range(DT):
                w2_cb = w2pool.tile([P, FT, P], F16, name="w2_cb")
                eng = nc.sync if dt % 2 == 0 else nc.gpsimd
                eng.dma_start(w2_cb[:], w2s[dt])
                for (o, s) in offs:
                    ps_o = po_pool.tile([P, 512], F32, name="ps_o")[:, :s]
                    for f in range(FT):
                        nc.tensor.matmul(ps_o[:], w2_cb[:, f, :],
                                         gts[f][:, o:o + s],
                                         start=(f == 0), stop=(f == FT - 1))
                    ob = opool.tile([P, 512], F32, name="ob")[:, :s]
                    nc.vector.tensor_copy(ob[:], ps_o[:])
                    nc.scalar.dma_start(outT[dt, :, o:o + s], ob[:])

    nc.finalize()
    return nc


_CACHE = {}
LAST_RESULTS = None


def _get_nc(C, D, F, chunks):
    key = (C, D, F, chunks)
    if key not in _CACHE:
        _CACHE[key] = build(C, D, F, chunks)
    return _CACHE[key]


def _split_chunks(C):
    """Split C into equal-ish pieces of at most 512 columns."""
    n = math.ceil(C / 512)
    base = C // n
    rem = C - base * n
    return tuple(base + (1 if i < rem else 0) for i in range(n))


def run_moe(hidden_states, router_w, w1, v1, w2):
    global LAST_RESULTS
    _install_trace_hook_if_requested()

    B, S, D = hidden_states.shape
    E = router_w.shape[1]
    F = w1.shape[2]
    T = B * S
    DC, FT, DT = D // P, F // P, D // P

    x = np.ascontiguousarray(hidden_states.reshape(T, D).astype(np.float32))

    # host router: fp64 logits; top-2 renormalized softmax = sigmoid of
    # the logit difference (full-softmax denominator cancels)
    logits = x.astype(np.float64) @ router_w.astype(np.float64)
    top1 = np.argmax(logits, axis=1)
    l1 = logits[np.arange(T), top1]
    masked = logits.copy()
    masked[np.arange(T), top1] = -np.inf
    top2 = np.argmax(masked, axis=1)
    l2 = masked[np.arange(T), top2]
    w_top1 = 1.0 / (1.0 + np.exp(l2 - l1))

    idx, wts = [], []
    for r in range(N_CORES):
        sel1 = top1 == r
        sel2 = top2 == r
        idx_r = np.where(sel1 | sel2)[0]
        we_r = np.where(sel1[idx_r], w_top1[idx_r], 1.0 - w_top1[idx_r])
        idx.append(idx_r)
        wts.append(we_r.astype(np.float32))

    C = max(256, math.ceil(max(len(i) for i in idx) / 8) * 8)
    chunks = _split_chunks(C)
    nc = _get_nc(C, D, F, chunks)

    in_maps = []
    for r in range(N_CORES):
        xg = np.zeros((C, D), dtype=np.float16)
        xg[:len(idx[r])] = x[idx[r]]
        xgT = np.ascontiguousarray(xg.T)
        # swizzle: wv[f, p, i, 0/1, j] = w1/v1[i*P+p, f*P+j]
        w1t = w1[r].astype(np.float16).reshape(DC, P, FT, P).transpose(2, 1, 0, 3)
        v1t = v1[r].astype(np.float16).reshape(DC, P, FT, P).transpose(2, 1, 0, 3)
        wvr = np.ascontiguousarray(np.stack([w1t, v1t], axis=3))
        # w2s[dt, p, i, j] = w2[i*P+p, dt*P+j]
        w2r = np.ascontiguousarray(
            w2[r].astype(np.float16).reshape(FT, P, DT, P).transpose(2, 1, 0, 3))
        in_maps.append({"xT": xgT, "wv": wvr, "w2s": w2r})

    res = run_bass_kernel_spmd(nc, in_maps, core_ids=list(range(N_CORES)))
    LAST_RESULTS = res

    out = np.zeros((T, D), dtype=np.float32)
    for r in range(N_CORES):
        o = res.results[r]["outT"].reshape(D, C)  # [D, C]
        n = len(idx[r])
        out[idx[r]] += wts[r][:, None] * o[:, :n].T
    return out.reshape(B, S, D)


def kernel(hidden_states, router_w, w1, v1, w2):
    return run_moe(hidden_states, router_w, w1, v1, w2)


# revision 8
# speedup vs baseline: 5.2918x; 1.0210x over previous
"""Sparse MoE routing kernel for trn2 (8 NeuronCores, expert-parallel).

Reference semantics:
    logits = x @ router_w; p = softmax(logits); top2 renormalized weights
    out[t] = sum_{e in top2(t)} we[t,e] * (silu(x@w1[e]) * (x@v1[e])) @ w2[e]

The reference computes every expert on every token densely, but only the
top-2 experts per token contribute to the output.  This kernel exploits
that sparsity: routing (a [T,8] matmul + top-2, 0.3% of the FLOPs) runs
on the host, which then gathers each expert's assigned tokens into a
compact padded buffer.  Core r holds expert r's weights and runs the
gated FFN only on its own tokens (~T/4 of them), in fp16 (PSUM
accumulation stays fp32; measured end-to-end rel err ~1e-3, tolerance
2e-2).  The host applies the per-token top-2 combine weights and
scatter-adds the per-expert partial outputs — each token appears in
exactly two experts' buffers, so no device collective is needed.

Per-core device work drops from 3 dense T-token matmuls (fp32r) to 3
C-token matmuls (fp16), C = max expert load padded to 128.
"""

import math
import os

import numpy as np

import concourse.mybir as mybir
import concourse.tile as tile
from concourse import bacc
from concourse.bass_utils import run_bass_kernel_spmd

P = 128
N_CORES = 8
F32 = mybir.dt.float32
F16 = mybir.dt.float16
ACTF = mybir.ActivationFunctionType


def _install_trace_hook_if_requested():
    """Optional: enables NTFF profiling when BASS_TRACE=1 (dev only)."""
    if os.environ.get("BASS_TRACE") != "1":
        return
    import sys
    import types

    if "antenv.axon_hooks" in sys.modules:
        return
    mod = types.ModuleType("antenv.axon_hooks")
    state = {"hook": None}
    mod.set_axon_ntff_profile_hook = lambda h: state.__setitem__("hook", h)
    mod.get_axon_ntff_profile_hook = lambda: state["hook"]
    sys.modules["antenv.axon_hooks"] = mod
    try:
        from trn_agent_boot.trn_boot import _ntff_profile_via_ctypes

        mod.set_axon_ntff_profile_hook(
            _ntff_profile_via_ctypes("/opt/axon/libaxon_pjrt.so")
        )
    except Exception:
        pass


def build(C, D, F, chunks):
    """Per-core program: gated FFN on C gathered tokens, fp16 matmuls.

    xT [D, C] fp16 (token on free axis); weights host-swizzled so every
    DMA line is per-partition contiguous:
      wv[f, p, d, 0/1, j]  = w1/v1[d*P+p, f*P+j]
      w2s[dt, p, f, j]     = w2[f*P+p, dt*P+j]
    outT [D, C] fp32 = (silu(w1.T x) * (v1.T x)) contracted with w2.
    `chunks` split C into <=512-column pieces (PSUM bank = 512 fp32).
    """
    DC = D // P   # contraction chunks over D
    FT = F // P   # partition tiles of F
    DT = D // P   # output d tiles
    assert sum(chunks) == C and all(s <= 512 for s in chunks)

    nc = bacc.Bacc("TRN2", target_bir_lowering=False, debug=False,
                   num_devices=N_CORES)

    xT = nc.dram_tensor("xT", [D, C], F16, kind="ExternalInput")
    wv = nc.dram_tensor("wv", [FT, P, DC, 2, P], F16, kind="ExternalInput")
    w2s = nc.dram_tensor("w2s", [DT, P, FT, P], F16, kind="ExternalInput")
    outT = nc.dram_tensor("outT", [DT, P, C], F32, kind="ExternalOutput")

    offs = []
    o = 0
    for s in chunks:
        offs.append((o, s))
        o += s

    with tile.TileContext(nc) as tc:
        with (
            tc.tile_pool(name="xpool", bufs=D // P) as xpool,
            tc.tile_pool(name="wpool", bufs=4) as wpool,
            tc.tile_pool(name="w2pool", bufs=2) as w2pool,
            tc.tile_pool(name="gpool", bufs=FT) as gpool,
            tc.tile_pool(name="spool", bufs=3) as spool,
            tc.tile_pool(name="opool", bufs=3) as opool,
            tc.tile_pool(name="ph", bufs=2, space="PSUM") as ph_pool,
            tc.tile_pool(name="pv", bufs=2, space="PSUM") as pv_pool,
            tc.tile_pool(name="po", bufs=3, space="PSUM") as po_pool,
        ):
            # gathered activations: all DC [P, C] tiles stay resident.
            # Column-chunk loads so the first accumulation group's slice
            # lands (and the PE starts) before the full x is resident.
            x_tiles = [xpool.tile([P, C], F16, name="x_sb")
                       for _ in range(DC)]
            engs = (nc.sync, nc.scalar, nc.gpsimd)

            def load_x_cols(o, s):
                for d in range(DC):
                    engs[d % 3].dma_start(x_tiles[d][:, o:o + s],
                                          xT[d * P:(d + 1) * P, o:o + s])

            def load_wv(f, nsplit):
                """Split the 1MB weight tile across several DMA rings so
                its latency shrinks when the rings are otherwise busy."""
                t = wpool.tile([P, DC, 2, P], F16, name="wv_cb")
                step = DC // nsplit
                for i in range(nsplit):
                    engs[(f + i) % 3].dma_start(
                        t[:, i * step:(i + 1) * step],
                        wv[f][:, i * step:(i + 1) * step])
                return t

            # critical path first: x columns for chunk 0, then the first
            # two weight tiles (split wide), then the rest of x
            load_x_cols(*offs[0])
            wv_pre = [load_wv(0, 4), load_wv(1, 4)]
            for (o, s) in offs[1:]:
                load_x_cols(o, s)

            # phase 1: g[f] = silu(w1.T x) * (v1.T x), kept in SBUF (fp16)
            gts = []
            for f in range(FT):
                wv_cb = wv_pre[f] if f < len(wv_pre) else load_wv(f, 2)
                gt = gpool.tile([P, C], F16, name="gt")
                for (o, s) in offs:
                    ps_h = ph_pool.tile([P, 512], F32, name="ps_h")[:, :s]
                    for d in range(DC):
                        nc.tensor.matmul(ps_h[:], wv_cb[:, d, 0, :],
                                         x_tiles[d][:, o:o + s],
                                         start=(d == 0), stop=(d == DC - 1))
                    ps_v = pv_pool.tile([P, 512], F32, name="ps_v")[:, :s]
                    for d in range(DC):
                        nc.tensor.matmul(ps_v[:], wv_cb[:, d, 1, :],
                                         x_tiles[d][:, o:o + s],
                                         start=(d == 0), stop=(d == DC - 1))
                    sl = spool.tile([P, 512], F16, name="sl")[:, :s]
                    nc.scalar.activation(sl[:], ps_h[:], ACTF.Silu)
                    nc.vector.tensor_mul(gt[:, o:o + s], sl[:], ps_v[:])
                gts.append(gt)

            # phase 2: outT[dt] = sum_f w2[f,dt].T g[f]
            for dt in range(DT):
                w2_cb = w2pool.tile([P, FT, P], F16, name="w2_cb")
                eng = nc.sync if dt % 2 == 0 else nc.gpsimd
                eng.dma_start(w2_cb[:], w2s[dt])
                for (o, s) in offs:
                    ps_o = po_pool.tile([P, 512], F32, name="ps_o")[:, :s]
                    for f in range(FT):
                        nc.tensor.matmul(ps_o[:], w2_cb[:, f, :],
                                         gts[f][:, o:o + s],
                                         start=(f == 0), stop=(f == FT - 1))
                    ob = opool.tile([P, 512], F32, name="ob")[:, :s]
                    nc.vector.tensor_copy(ob[:], ps_o[:])
                    nc.scalar.dma_start(outT[dt, :, o:o + s], ob[:])

    nc.finalize()
    return nc


_CACHE = {}
LAST_RESULTS = None


def _get_nc(C, D, F, chunks):
    key = (C, D, F, chunks)
    if key not in _CACHE:
        _CACHE[key] = build(C, D, F, chunks)
    return _CACHE[key]


def _split_chunks(C):
    """Split C into equal-ish pieces of at most 512 columns."""
    n = math.ceil(C / 512)
    base = C // n
    rem = C - base * n
    return tuple(base + (1 if i < rem else 0) for i in range(n))


def run_moe(hidden_states, router_w, w1, v1, w2):
    global LAST_RESULTS
    _install_trace_hook_if_requested()

    B, S, D = hidden_states.shape
    E = router_w.shape[1]
    F = w1.shape[2]
    T = B * S
    DC, FT, DT = D // P, F // P, D // P

    x = np.ascontiguousarray(hidden_states.reshape(T, D).astype(np.float32))

    # host router: fp64 logits; top-2 renormalized softmax = sigmoid of
    # the logit difference (full-softmax denominator cancels)
    logits = x.astype(np.float64) @ router_w.astype(np.float64)
    top1 = np.argmax(logits, axis=1)
    l1 = logits[np.arange(T), top1]
    masked = logits.copy()
    masked[np.arange(T), top1] = -np.inf
    top2 = np.argmax(masked, axis=1)
    l2 = masked[np.arange(T), top2]
    w_top1 = 1.0 / (1.0 + np.exp(l2 - l1))

    idx, wts = [], []
    for r in range(N_CORES):
        sel1 = top1 == r
        sel2 = top2 == r
        idx_r = np.where(sel1 | sel2)[0]
        we_r = np.where(sel1[idx_r], w_top1[idx_r], 1.0 - w_top1[idx_r])
        idx.append(idx_r)
        wts.append(we_r.astype(np.float32))

    C = max(256, math.ceil(max(len(i) for i in idx) / 8) * 8)
    chunks = _split_chunks(C)
    nc = _get_nc(C, D, F, chunks)

    in_maps = []
    for r in range(N_CORES):
        xg = np.zeros((C, D), dtype=np.float16)
        xg[:len(idx[r])] = x[idx[r]]
        xgT = np.ascontiguousarray(xg.T)
        # swizzle: wv[f, p, i, 0/1, j] = w1/v1[i*P+p, f*P+j]
        w1t = w1[r].astype(np.float16).reshape(DC, P, FT, P).transpose(2, 1, 0, 3)
        v1t = v1[r].astype(np.float16).reshape(DC, P, FT, P).transpose(2, 1, 0, 3)
        wvr = np.ascontiguousarray(np.stack([w1t, v1t], axis=3))
        # w2s[dt, p, i, j] = w2[i*P+p, dt*P+j]
        w2r = np.ascontiguousarray(
            w2[r].astype(np.float16).reshape(FT, P, DT, P).transpose(2, 1, 0, 3))
        in_maps.append({"xT": xgT, "wv": wvr, "w2s": w2r})

    res = run_bass_kernel_spmd(nc, in_maps, core_ids=list(range(N_CORES)))
    LAST_RESULTS = res

    out = np.zeros((T, D), dtype=np.float32)
    for r in range(N_CORES):
        o = res.results[r]["outT"].reshape(D, C)  # [D, C]
        n = len(idx[r])
        out[idx[r]] += wts[r][:, None] * o[:, :n].T
    return out.reshape(B, S, D)


def kernel(hidden_states, router_w, w1, v1, w2):
    return run_moe(hidden_states, router_w, w1, v1, w2)
